# revision 29
# baseline (speedup 1.0000x reference)
"""Single-head attention (B=4, T=4096, C=1024, H=64) on 8 trn2 NeuronCores.

Sharding: 8 shards = (batch b, query-half h).  Each core receives x[b]
pre-transposed to xT [C=1024, T=4096] in bf16; for h==1 the T columns are
rotated by 2048 so that "this core's" 2048 queries are always columns 0:2048
(softmax is permutation-invariant over keys).  SPMD program identical on all
cores, no rank logic.

v2 (vs the 126us baseline): the trace showed PE ~95% busy (the real
bottleneck) while the ACT exp stream had 40us of idle.  Changes:

  * scores matmuls (contraction = H = 64) now run ROW-TILED: two
    independent 64x128 PE tiles (T0 rows 0:63, T8 rows 64:127) compute two
    key tiles' scores concurrently into the two banks of one PSUM chunk
    [128, 2, 512].  Key tiles are paired across block pairs: EVEN blocks
    project with [Wk|Wv] (K^T in PSUM rows 0:64, V^T in 64:128), ODD
    blocks with [Wv|Wk], so pair p = 4i+j is (tile j of block 2i) at SBUF
    partitions 0:64 + (tile j of block 2i+1) at partitions 64:128.  Q^T is
    replicated into both partition halves by projecting with [Wq|Wq].
  * attn@V runs in fp8e4 (e4m3) with perf_mode=DoubleRow: virtual
    contraction 256 = both key tiles of a pair in ONE matmul (2 instead of
    4 MMs per unit).  exp output is written as fp8 planes ex[128, 2, 1024];
    V_aug (V columns + ones column for the softmax denominator) is stored
    fp8 as va[128, 16, 2, 80].
  * V tiles are transposed with a plain matmul against a 64x64 identity
    (V = (V^T).T @ I), row-tiled — cheaper than transpose-mode.
  * ramp: blocks 0+1 + q0/q1 are DMA'd/projected inline ahead of the
    pipelined loop; everything else dribbles in as fillers.
  * epilogue unchanged: raw [65, 512] num/den slabs out, host divides.

Steady state is ACT-bound: 64 exp chunks x ~1.11us = ~71us.
"""

import os
import sys

for _p in ("/opt/trn_rl_repo", "/root/.axon_site/_ro/trn_rl_repo"):
    if os.path.isdir(_p) and _p not in sys.path:
        sys.path.append(_p)

from collections import deque

import numpy as np

import concourse.bacc as bacc
import concourse.mybir as mybir
import concourse.tile as tile
from concourse.bass_utils import run_bass_kernel_spmd

B = 4
T = 4096
C = 1024
H = 64
TQ = T // 2  # queries per core
N_CORES = 8

F32 = mybir.dt.float32
BF16 = mybir.dt.bfloat16
FP8 = mybir.dt.float8e4

NC_CH = C // 128  # 8 contraction chunks
NSB = T // 512  # 8 key/source blocks of 512
NST = T // 128  # 32 key tiles of 128
NPAIR = NST // 2  # 16 row-tiled key-tile pairs
NQB = TQ // 512  # 4 query blocks of 512

AV_FP8 = False  # fp8e4 DoubleRow attn@V (fallback: bf16, 4 MMs/unit)

DR = mybir.MatmulPerfMode.DoubleRow
EXP = mybir.ActivationFunctionType.Exp


def _build_module():
    nc = bacc.Bacc("TRN2", target_bir_lowering=False, debug=False, num_devices=N_CORES)

    xt_d = nc.dram_tensor("xt", [2 * NSB, 128, 4, 512], BF16, kind="ExternalInput").ap()
    wkv_d = nc.dram_tensor("wkv", [128, NC_CH, 128], BF16, kind="ExternalInput").ap()
    wvk_d = nc.dram_tensor("wvk", [128, NC_CH, 128], BF16, kind="ExternalInput").ap()
    wq_d = nc.dram_tensor("wq", [128, NC_CH, 128], BF16, kind="ExternalInput").ap()
    ident_d = nc.dram_tensor("ident", [128, 64], BF16, kind="ExternalInput").ap()
    out = nc.dram_tensor("out", [NQB, 65, 512], F32, kind="ExternalOutput").ap()

    ex_dt = FP8 if AV_FP8 else BF16

    with tile.TileContext(nc) as tc:
        with (
            tc.tile_pool(name="const", bufs=1) as const_pool,
            tc.tile_pool(name="xt", bufs=8) as xt_pool,
            tc.tile_pool(name="big", bufs=1) as big_pool,
            tc.tile_pool(name="vstage", bufs=2) as vstage_pool,
            tc.tile_pool(name="exp", bufs=3) as exp_pool,
            tc.tile_pool(name="outts", bufs=2) as outts_pool,
            tc.tile_pool(name="p1", bufs=2, space="PSUM") as psum_p1,
            tc.tile_pool(name="psc", bufs=2, space="PSUM") as psum_sc,
            tc.tile_pool(name="pacc", bufs=2, space="PSUM") as psum_acc,
        ):
            # ---- constants ----
            wkv_sb = const_pool.tile([128, NC_CH, 128], BF16, tag="wkv")
            wvk_sb = const_pool.tile([128, NC_CH, 128], BF16, tag="wvk")
            wq_sb = const_pool.tile([128, NC_CH, 128], BF16, tag="wq")
            # 64x64 identity in both partition halves (rhs of the
            # matmul-transpose; T8 reads rows 64:128, T0 rows 0:64).
            ident_bf = const_pool.tile([128, 64], BF16, tag="ident_bf")

            # ---- persistent activations ----
            # kt[0:64, p, :]   = K^T of tile a_p = 8*(p//4) + p%4       (even block)
            # kt[64:128, p, :] = K^T of tile b_p = 8*(p//4) + 4 + p%4   (odd block)
            kt_sb = big_pool.tile([128, NPAIR, 128], BF16, tag="kt")
            # Q^T replicated: qt[0:64] == qt[64:128]
            qt_sb = big_pool.tile([128, TQ], BF16, tag="qt")
            # V_aug per pair: va[:, p, 0, 0:65] = [V(a_p) | 1], plane 1 = b_p.
            # inner stride 80 bytes keeps the DoubleRow Ko-step 16B-aligned.
            va = big_pool.tile([128, NPAIR, 2, 80], ex_dt, tag="va")
            nc.gpsimd.memset(va[:, :, :, 64:65], 1.0)

            dma_engines = (nc.sync, nc.gpsimd)
            dma_i = [0]

            def next_dma():
                e = dma_engines[dma_i[0] % len(dma_engines)]
                dma_i[0] += 1
                return e

            # ---- phase 1 pieces ----
            def emit_xdma(sb, quarters=False):
                xt = xt_pool.tile([128, NC_CH, 512], BF16, tag="xt", name=f"xt{sb}")
                if quarters:
                    for q in range(4):
                        next_dma().dma_start(
                            xt[:, 2 * q : 2 * q + 2, :],
                            xt_d[2 * sb + q // 2][:, 2 * (q % 2) : 2 * (q % 2) + 2, :],
                        )
                else:
                    for half in range(2):
                        next_dma().dma_start(
                            xt[:, 4 * half : 4 * half + 4, :], xt_d[2 * sb + half]
                        )
                return xt

            def emit_kv_mm(sb, xt, kv_ps, c0, c1, k0=0, k1=512):
                w = wkv_sb if sb % 2 == 0 else wvk_sb
                for c in range(c0, c1):
                    nc.tensor.matmul(
                        kv_ps[:, 0 : k1 - k0],
                        w[:, c, :],
                        xt[:, c, k0:k1],
                        start=(c == 0),
                        stop=(c == NC_CH - 1),
                    )

            def emit_kv_copies(sb, kv_ps):
                i = sb // 2
                if sb % 2 == 0:  # K^T rows 0:64, V^T rows 64:128
                    nc.vector.tensor_copy(kt_sb[0:64, 4 * i : 4 * i + 4, :], kv_ps[0:64, :])
                    vt_sb = vstage_pool.tile([128, 512], BF16, tag="vst", name=f"vt{sb}")
                    nc.vector.tensor_copy(vt_sb[64:128, :], kv_ps[64:128, :])
                else:  # K^T rows 64:128, V^T rows 0:64
                    nc.vector.tensor_copy(
                        kt_sb[64:128, 4 * i : 4 * i + 4, :], kv_ps[64:128, :]
                    )
                    vt_sb = vstage_pool.tile([128, 512], BF16, tag="vst", name=f"vt{sb}")
                    nc.vector.tensor_copy(vt_sb[0:64, :], kv_ps[0:64, :])
                return vt_sb

            def emit_q_mm(sb, xt, q_ps, c0, c1):
                for c in range(c0, c1):
                    nc.tensor.matmul(
                        q_ps[:],
                        wq_sb[:, c, :],
                        xt[:, c, :],
                        start=(c == 0),
                        stop=(c == NC_CH - 1),
                    )

            def emit_v(sb, vt_sb, j0, j1):
                # V tile j of block sb -> va plane (sb%2) of pair 4*(sb//2)+j,
                # via plain matmul (V^T).T @ I on the row tile matching where
                # V^T sits (even block: partitions 64:128 -> T8; odd: T0).
                i, half = divmod(sb, 2)
                lo, hi = (64, 128) if half == 0 else (0, 64)
                for j in range(j0, j1):
                    p = 4 * i + j
                    v_ps = psum_p1.tile([128, 64], F32, tag="p1", name=f"v{sb}_{j}")
                    nc.tensor.matmul(
                        v_ps[:],
                        vt_sb[lo:hi, j * 128 : (j + 1) * 128],
                        ident_bf[lo:hi, :],
                        start=True,
                        stop=True,
                    )
                    nc.vector.tensor_copy(va[:, p, half, 0:64], v_ps[:])

            def kt_row(sb):
                return (0, 64) if sb % 2 == 0 else (64, 128)

            # filler queue: small PE chunks drained between attention chunks.
            filler = deque()
            proj_state = {}

            def queue_kv_block(sb, xt=None):
                state = proj_state.setdefault(sb, {})
                if xt is not None:
                    state["xt"] = xt

                def dma_piece(_sb=sb):
                    state["xt"] = emit_xdma(_sb)

                def kv_mm(c0, c1, _sb=sb):
                    if "kv" not in state:
                        state["kv"] = psum_p1.tile(
                            [128, 512], F32, tag="p1", name=f"kv{_sb}"
                        )
                    emit_kv_mm(_sb, state["xt"], state["kv"], c0, c1)

                def copies(_sb=sb):
                    state["vt"] = emit_kv_copies(_sb, state["kv"])

                if xt is None:
                    filler.append(dma_piece)
                for c0 in range(0, NC_CH, 2):
                    filler.append(lambda c0=c0: kv_mm(c0, c0 + 2))
                filler.append(copies)
                filler.append(lambda _sb=sb: emit_v(_sb, proj_state[_sb]["vt"], 0, 2))
                filler.append(lambda _sb=sb: emit_v(_sb, proj_state[_sb]["vt"], 2, 4))

            def queue_q_block(sb):
                state = proj_state[sb]

                def q_mm(c0, c1, _sb=sb):
                    if "q" not in state:
                        state["q"] = psum_p1.tile(
                            [128, 512], F32, tag="p1", name=f"q{_sb}"
                        )
                    emit_q_mm(_sb, state["xt"], state["q"], c0, c1)

                def q_copy(_sb=sb):
                    nc.vector.tensor_copy(
                        qt_sb[:, _sb * 512 : (_sb + 1) * 512], state["q"][:]
                    )

                for c0 in range(0, NC_CH, 2):
                    filler.append(lambda c0=c0: q_mm(c0, c0 + 2))
                filler.append(q_copy)

            # ---- phase 2: software-pipelined attention ----
            # unit u = (tcp, p): key-tile pair p against queries tcp*1024:+1024.
            # chunk m = 2*u + k: pair p x queries k*512:(k+1)*512, one PSUM
            # chunk [128, 2, 512] (T0 -> plane 0 = tile a_p, T8 -> plane 1 =
            # tile b_p, concurrently) and one [128, (2,512)] exp.
            units = [(0, p) for p in range(NPAIR)] + [(1, p) for p in range(NPAIR)]
            NCHUNK = 2 * len(units)
            sc_tiles = {}
            ex_tiles = {}
            outt_tiles = {}

            def emit_sc(m):
                u, k = divmod(m, 2)
                tcp, p = units[u]
                sc_ps = psum_sc.tile([128, 2, 512], F32, tag="sc", name=f"sc{m}")
                sc_tiles[m] = sc_ps
                q0 = tcp * 1024 + k * 512
                nc.tensor.matmul(
                    sc_ps[:, 0, :],
                    kt_sb[0:64, p, :],
                    qt_sb[0:64, q0 : q0 + 512],
                    start=True,
                    stop=True,
                    tile_position=(0, 0),
                )
                nc.tensor.matmul(
                    sc_ps[:, 1, :],
                    kt_sb[64:128, p, :],
                    qt_sb[64:128, q0 : q0 + 512],
                    start=True,
                    stop=True,
                    tile_position=(64, 0),
                )

            def get_ex(u):
                if u not in ex_tiles:
                    ex_tiles[u] = exp_pool.tile(
                        [128, 2, 1024], ex_dt, tag="exp", name=f"ex{u}"
                    )
                return ex_tiles[u]

            def get_outt(tcp):
                if tcp not in outt_tiles:
                    oa = psum_acc.tile([65, 512], F32, tag="acc", name=f"outt_a{tcp}")
                    ob = psum_acc.tile([65, 512], F32, tag="acc", name=f"outt_b{tcp}")
                    outt_tiles[tcp] = (oa, ob)
                return outt_tiles[tcp]

            def emit_act(m):
                u, k = divmod(m, 2)
                sc_ps = sc_tiles.pop(m)
                ex = get_ex(u)
                nc.scalar.activation(
                    ex[:, :, k * 512 : (k + 1) * 512], sc_ps[:], EXP, scale=0.125
                )

            def emit_av(u):
                tcp, p = units[u]
                ex = ex_tiles.pop(u)
                outts = get_outt(tcp)
                if AV_FP8:
                    for i, outt_ps in enumerate(outts):
                        nc.tensor.matmul(
                            outt_ps[:],
                            va[:, p, :, 0:65],
                            ex[:, :, i * 512 : (i + 1) * 512],
                            start=(p == 0),
                            stop=(p == NPAIR - 1),
                            perf_mode=DR,
                        )
                else:
                    # plane-major so each va weight tile is loaded once per
                    # two matmuls.
                    for j in range(2):
                        for i, outt_ps in enumerate(outts):
                            nc.tensor.matmul(
                                outt_ps[:],
                                va[:, p, j, 0:65],
                                ex[:, j, i * 512 : (i + 1) * 512],
                                start=(p == 0 and j == 0),
                                stop=(p == NPAIR - 1 and j == 1),
                            )

            def emit_epilogue(tcp):
                for i, outt_ps in enumerate(outt_tiles[tcp]):
                    tci = 2 * tcp + i
                    outt_sb = outts_pool.tile([65, 512], F32, tag="outts", name=f"os{tci}")
                    if tcp == 1 and i == 0:
                        # final epilogue: scalar engine is done with exp, use
                        # it for one copy so the two slabs evacuate in parallel
                        nc.scalar.copy(outt_sb[:], outt_ps[:])
                    else:
                        nc.vector.tensor_copy(outt_sb[:], outt_ps[:])
                    (nc.sync if i == 0 else nc.gpsimd).dma_start(out[tci], outt_sb[:])

            # ---- ramp DMA: 16 single-chunk pieces engage all HW queues ----
            xt0 = xt_pool.tile([128, NC_CH, 512], BF16, tag="xt", name="xt0")
            xt1 = xt_pool.tile([128, NC_CH, 512], BF16, tag="xt", name="xt1")

            def xpiece(eng, xt, sb, c):
                eng.dma_start(
                    xt[:, c : c + 1, :], xt_d[2 * sb + c // 4][:, c % 4 : c % 4 + 1, :]
                )

            # Critical-path loads go on the SYNC queue: it boots ~2us before
            # gpsimd and, unlike scalar, is not blocked by the ~1.3us
            # ACT_TABLE_LOAD that walrus schedules first on the scalar queue.
            nc.sync.dma_start(wkv_sb[:], wkv_d)
            nc.sync.dma_start(wq_sb[:], wq_d)
            for c in range(4):
                xpiece(nc.sync, xt0, 0, c)
            nc.gpsimd.dma_start(wvk_sb[:], wvk_d)
            for c in range(4, NC_CH):
                xpiece(nc.gpsimd, xt0, 0, c)
            for c in range(4):
                xpiece(nc.gpsimd, xt1, 1, c)
            for c in range(4, NC_CH):
                xpiece(nc.scalar, xt1, 1, c)
            nc.scalar.dma_start(ident_bf[:], ident_d)

            # ---- ramp compute: pair 0 + q0/q1 on the shortest path ----
            # kv/q matmuls interleave per x chunk so the PE follows the DMA.
            vt0 = vstage_pool.tile([128, 512], BF16, tag="vst", name="vt0")
            vt1 = vstage_pool.tile([128, 512], BF16, tag="vst", name="vt1")

            def ramp_proj(sb, xt, vt, qoff):
                # key-tile 0 columns (pair 0) + full q block, chunk-interleaved
                kva = psum_p1.tile([128, 128], F32, tag="p1", name=f"kv{sb}a")
                qp = psum_p1.tile([128, 512], F32, tag="p1", name=f"q{sb}")
                w = wkv_sb if sb % 2 == 0 else wvk_sb
                for c in range(NC_CH):
                    nc.tensor.matmul(
                        kva[:], w[:, c, :], xt[:, c, 0:128],
                        start=(c == 0), stop=(c == NC_CH - 1),
                    )
                    nc.tensor.matmul(
                        qp[:], wq_sb[:, c, :], xt[:, c, :],
                        start=(c == 0), stop=(c == NC_CH - 1),
                    )
                if sb % 2 == 0:
                    nc.vector.tensor_copy(kt_sb[0:64, 0:1, :], kva[0:64, :])
                    nc.vector.tensor_copy(vt[64:128, 0:128], kva[64:128, :])
                else:
                    nc.vector.tensor_copy(kt_sb[64:128, 0:1, :], kva[64:128, :])
                    nc.vector.tensor_copy(vt[0:64, 0:128], kva[0:64, :])
                nc.vector.tensor_copy(qt_sb[:, qoff : qoff + 512], qp[:])

            ramp_proj(0, xt0, vt0, 0)
            ramp_proj(1, xt1, vt1, 512)
            emit_sc(0)
            emit_sc(1)
            emit_v(0, vt0, 0, 1)
            emit_v(1, vt1, 0, 1)
            # pair 1 key columns (needed by sc(2)/sc(3), emitted at m=0/1)
            for sb, xt in ((0, xt0), (1, xt1)):
                kvb = psum_p1.tile([128, 128], F32, tag="p1", name=f"kv{sb}b1")
                emit_kv_mm(sb, xt, kvb, 0, NC_CH, 128, 256)
                lo, hi = kt_row(sb)
                nc.vector.tensor_copy(kt_sb[lo:hi, 1:2, :], kvb[lo:hi, :])
                vt = (vt0, vt1)[sb]
                vlo, vhi = (64, 128) if sb == 0 else (0, 64)
                nc.vector.tensor_copy(vt[vlo:vhi, 128:256], kvb[vlo:vhi, :])

            proj_state[0] = {"xt": xt0, "vt": vt0}
            proj_state[1] = {"xt": xt1, "vt": vt1}
            # Blocks 2-7's x DMA is time-gated (tile_wait_until) on the sync
            # queue so it doesn't race blocks 0/1 for bandwidth during the
            # ramp; staggered so arrival roughly tracks consumption order.
            for sb in range(2, NSB):
                with tc.tile_wait_until(0.012 + 0.0025 * (sb // 2 - 1)):
                    xt_g = xt_pool.tile(
                        [128, NC_CH, 512], BF16, tag="xt", name=f"xt{sb}"
                    )
                    for half in range(2):
                        nc.sync.dma_start(
                            xt_g[:, 4 * half : 4 * half + 4, :], xt_d[2 * sb + half]
                        )
                    proj_state[sb] = {"xt": xt_g}

            # remaining key columns of blocks 0/1 (pairs 2, 3) as fillers
            def kv01_rest(sb, p):
                state = proj_state[sb]
                xt = state["xt"]
                kvb = psum_p1.tile([128, 128], F32, tag="p1", name=f"kv{sb}b{p}")
                emit_kv_mm(sb, xt, kvb, 0, NC_CH, p * 128, (p + 1) * 128)
                lo, hi = kt_row(sb)
                nc.vector.tensor_copy(kt_sb[lo:hi, p : p + 1, :], kvb[lo:hi, :])
                vt = state["vt"]
                vlo, vhi = (64, 128) if sb % 2 == 0 else (0, 64)
                nc.vector.tensor_copy(vt[vlo:vhi, p * 128 : (p + 1) * 128], kvb[vlo:vhi, :])

            def kv_half(sb, h):
                state = proj_state[sb]
                if "kv" not in state:
                    state["kv"] = psum_p1.tile([128, 512], F32, tag="p1", name=f"kv{sb}")
                emit_kv_mm(sb, state["xt"], state["kv"], 4 * h, 4 * h + 4)

            def kv_cp(sb):
                proj_state[sb]["vt"] = emit_kv_copies(sb, proj_state[sb]["kv"])

            def vf(sb, j0, j1):
                emit_v(sb, proj_state[sb]["vt"], j0, j1)

            def vf1(sb, j):
                emit_v(sb, proj_state[sb]["vt"], j, j + 1)

            pad = lambda: None

            # Interleaved filler schedule, 3/chunk for m<6 then 2/chunk.
            # Invariants: (a) no chunk gets more than ~1us of PE filler
            # (bunched kv bursts head-of-line blocked the exp chain ~3us),
            # (b) at most ONE emit_v psum tile is allocated while a kv/q
            # projection tile is live (p1 pool has only 2 buffers).
            F = filler.append
            F(lambda: kv01_rest(0, 2)); F(lambda: kv01_rest(1, 2)); F(lambda: emit_v(0, vt0, 1, 2))   # m0
            F(lambda: kv01_rest(0, 3)); F(lambda: kv01_rest(1, 3)); F(lambda: emit_v(1, vt1, 1, 2))   # m1
            F(lambda: emit_v(0, vt0, 2, 3)); F(lambda: emit_v(1, vt1, 2, 3)); F(lambda: emit_v(0, vt0, 3, 4))  # m2
            F(lambda: emit_v(1, vt1, 3, 4)); F(lambda: kv_half(2, 0)); F(lambda: kv_half(2, 1))       # m3
            F(lambda: kv_cp(2)); F(lambda: kv_half(3, 0)); F(lambda: kv_half(3, 1))                   # m4
            F(lambda: kv_cp(3)); F(lambda: vf1(2, 0)); F(lambda: vf1(2, 1))                           # m5
            F(lambda: vf1(2, 2)); F(lambda: vf1(2, 3))    # m6
            F(lambda: vf1(3, 0)); F(lambda: vf1(3, 1))    # m7
            F(lambda: vf1(3, 2)); F(lambda: vf1(3, 3))    # m8
            F(lambda: kv_half(4, 0)); F(pad)              # m9
            F(lambda: kv_half(4, 1)); F(lambda: kv_cp(4))  # m10
            F(lambda: vf1(4, 0)); F(lambda: vf1(4, 1))    # m11
            F(lambda: kv_half(5, 0)); F(lambda: vf1(4, 2))  # m12
            F(lambda: kv_half(5, 1)); F(lambda: kv_cp(5))  # m13
            F(lambda: vf1(4, 3)); F(lambda: vf1(5, 0))    # m14
            F(lambda: kv_half(6, 0)); F(lambda: vf1(5, 1))  # m15
            F(lambda: kv_half(6, 1)); F(lambda: kv_cp(6))  # m16
            F(lambda: vf1(5, 2)); F(lambda: vf1(5, 3))    # m17
            F(lambda: kv_half(7, 0)); F(lambda: vf1(6, 0))  # m18
            F(lambda: kv_half(7, 1)); F(lambda: kv_cp(7))  # m19
            F(lambda: vf1(6, 1)); F(lambda: vf1(7, 0))    # m20
            F(lambda: vf1(6, 2)); F(lambda: vf1(7, 1))    # m21
            F(lambda: vf1(6, 3)); F(lambda: vf1(7, 2))    # m22
            F(lambda: vf1(7, 3)); F(pad)                  # m23
            queue_q_block(2)                              # m24-26 (4 pieces + copy)
            F(pad)
            queue_q_block(3)                              # m27-29
            F(pad)

            # ---- pipelined loop: sc runs 2 chunks ahead of exp ----
            for m in range(NCHUNK):
                emit_act(m)
                if m + 2 < NCHUNK:
                    emit_sc(m + 2)
                for _ in range(3 if m < 6 else 2):
                    if filler:
                        filler.popleft()()
                if m % 2 == 1:
                    emit_av((m - 1) // 2)
                    if (m - 1) // 2 == NPAIR - 1:
                        emit_epilogue(0)
            emit_epilogue(1)

    nc.compile()
    return nc


_NC_CACHE = None


def _get_module():
    global _NC_CACHE
    if _NC_CACHE is None:
        _NC_CACHE = _build_module()
    return _NC_CACHE


def _make_in_maps(x, Wq, Wk, Wv):
    import ml_dtypes

    bf16 = ml_dtypes.bfloat16
    xT = np.transpose(np.asarray(x, dtype=np.float32), (0, 2, 1)).astype(bf16)  # [B,C,T]
    wq = np.asarray(Wq, dtype=np.float32)
    wk = np.asarray(Wk, dtype=np.float32)
    wv = np.asarray(Wv, dtype=np.float32)

    def chunked(w):  # [1024, 128] -> [128, 8, 128]
        return np.ascontiguousarray(
            w.reshape(NC_CH, 128, 128).transpose(1, 0, 2).astype(bf16)
        )

    wkv = chunked(np.concatenate([wk, wv], axis=1))
    wvk = chunked(np.concatenate([wv, wk], axis=1))
    wq2 = chunked(np.concatenate([wq, wq], axis=1))
    ident = np.ascontiguousarray(
        np.concatenate([np.eye(64), np.eye(64)], axis=0).astype(bf16)
    )
    in_maps = []
    for core in range(N_CORES):
        b, h = divmod(core, 2)
        xb = xT[b]
        if h == 1:
            xb = np.concatenate([xb[:, TQ:], xb[:, :TQ]], axis=1)
        xb = (
            xb.reshape(2, 4, 128, NSB, 512)
            .transpose(3, 0, 2, 1, 4)
            .reshape(2 * NSB, 128, 4, 512)
        )
        in_maps.append(
            {
                "xt": np.ascontiguousarray(xb),
                "wkv": wkv,
                "wvk": wvk,
                "wq": wq2,
                "ident": ident,
            }
        )
    return in_maps


def _unshard(results):
    out = np.empty((B, T, H), dtype=np.float32)
    for core in range(N_CORES):
        b, h = divmod(core, 2)
        slab = results[core]["out"]  # [NQB, 65, 512]
        num = slab[:, 0:64, :]
        den = slab[:, 64:65, :]
        o = (num / den).transpose(0, 2, 1).reshape(TQ, H)
        out[b, h * TQ : (h + 1) * TQ, :] = o
    return out


def run(x, Wq, Wk, Wv, **spmd_kwargs):
    """Run on hardware; returns (output, BassKernelResults)."""
    nc = _get_module()
    in_maps = _make_in_maps(x, Wq, Wk, Wv)
    res = run_bass_kernel_spmd(nc, in_maps, core_ids=list(range(N_CORES)), **spmd_kwargs)
    return _unshard(res.results), res


def kernel(x, Wq, Wk, Wv):
    out, _ = run(x, Wq, Wk, Wv)
    return out
